# revision 14
# baseline (speedup 1.0000x reference)
"""Attention-pooling kernel for Trainium2, data-parallel over batch on 8 NeuronCores.

Reference computation per batch n:
  vf = v[n].reshape(L, C)
  x  = concat([vf, broadcast(h[n])], axis=1)      # [L, C+Dh]
  e  = tanh(x @ W1.T + b1)                        # [L, Dh]
  s  = e @ W2.T + b2                              # [L, 1]
  a  = softmax(s, axis=0)
  out[n] = sum_l a[l] * vf[l]                     # [C]

Implementation notes:
  - Sharded batch-parallel: 64 batches -> 8 cores x 8 batches, weights replicated.
  - W1 split: x @ W1.T = vf @ W1v.T + h[n] @ W1h.T; the h-term ("g" = W1h h + b1)
    is computed once per batch instead of once per (batch, l) — halves the flops.
  - b2 is dropped: softmax is shift-invariant and b2 cancels exactly.
  - e is produced in natural [l, d] layout (lhsT=vfT, rhs=w1vT); g is DVE-preloaded
    into PSUM and the matmuls accumulate on top, so tanh needs no bias.
  - scores = sum_d w2[d]*e[l, d] is a free-dim weighted reduction on DVE, landing
    directly in the [l(p), lo] layout pooling wants — no score/pT work on the PE.
  - Softmax without max-subtraction: scores are tiny here and exp stays well inside
    f32 range; normalization is folded into the output copy.
  - v is cast to bf16 once; vfT (c-on-partitions) made by xbar DMA transpose in
    steady state, by the TensorEngine for the first two batches (prologue is
    DMA-bound, PE idle).
"""

import os
from contextlib import ExitStack

import numpy as np

import concourse.bass as bass
import concourse.mybir as mybir
import concourse.tile as tile
from concourse import bacc
from concourse.bass_utils import run_bass_kernel_spmd
from concourse.masks import make_identity

N_CORES = 8
N = 64
NB = N // N_CORES  # batches per core
P = 128
L = 1024  # spatial 32*32
C = 1024  # v channels
DH = 1024  # hidden
F = C + DH
FP32 = mybir.dt.float32
BF16 = mybir.dt.bfloat16
TANH = mybir.ActivationFunctionType.Tanh
EXP = mybir.ActivationFunctionType.Exp

LO = L // P
CO = C // P
DO = DH // P
FO = F // P


def build_kernel(nb: int = NB) -> bass.Bass:
    """Build the per-core Bass graph for an nb-batch shard."""
    nc = bacc.Bacc("TRN2", target_bir_lowering=False, debug=False)

    h_ext = nc.declare_dram_parameter("h", [nb, DH], FP32, isOutput=False)
    v_ext = nc.declare_dram_parameter("v", [nb, L, C], FP32, isOutput=False)
    w1_ext = nc.declare_dram_parameter("W1", [DH, F], FP32, isOutput=False)
    b1_ext = nc.declare_dram_parameter("b1", [DH], FP32, isOutput=False)
    w2_ext = nc.declare_dram_parameter("W2", [1, DH], FP32, isOutput=False)
    out_ext = nc.declare_dram_parameter("out", [nb, C], FP32, isOutput=True)

    gscr = nc.dram_tensor("gscr", [nb, DH], FP32)

    def bcast(ap2d, parts):
        """Partition-broadcast AP for a 1D/row DRAM access pattern."""
        ap = ap2d.ap if len(ap2d.shape) == 1 else ap2d.ap[1:]
        return bass.AP(tensor=ap2d.tensor, offset=ap2d.offset, ap=[[0, parts], *ap])

    with tile.TileContext(nc) as tc, ExitStack() as ctx:
        consts = ctx.enter_context(tc.tile_pool(name="consts", bufs=1))
        wstage = ctx.enter_context(tc.tile_pool(name="wstage", bufs=2))
        vpool = ctx.enter_context(tc.tile_pool(name="vpool", bufs=2))
        spool = ctx.enter_context(tc.tile_pool(name="spool", bufs=3))
        npool = ctx.enter_context(tc.tile_pool(name="npool", bufs=2))
        mpsum = ctx.enter_context(tc.tile_pool(name="mpsum", bufs=3, space="PSUM"))
        tpsum = ctx.enter_context(tc.tile_pool(name="tpsum", bufs=1, space="PSUM"))
        spsum = ctx.enter_context(tc.tile_pool(name="spsum", bufs=1, space="PSUM"))

        identity = consts.tile([P, P], BF16)
        make_identity(nc, identity)

        # ---- phase A for n=0, hoisted so v[0] streams in before W1 prep ----
        # For the first two batches the transpose runs on the (otherwise idle)
        # TensorEngine so the prologue DMA stream stays in pure copy mode; from
        # n=2 on, the xbar DMA transpose overlaps fully under the MLP matmuls.
        def phase_a(n):
            vf_bf = vpool.tile([P, LO, C], BF16, tag="vf", name=f"vf_{n}")
            for lo in range(LO):
                st = spool.tile([P, C], FP32, tag="vstage", name=f"vst_{n}_{lo}")
                nc.sync.dma_start(st, v_ext[n, bass.ts(lo, P), :])
                nc.scalar.copy(vf_bf[:, lo, :], st)
            vfT_bf = vpool.tile([P, CO, L], BF16, tag="vfT", name=f"vfT_{n}")
            if n < 2:
                for lo in range(LO):
                    for co in range(CO):
                        pt = tpsum.tile(
                            [P, P], BF16, tag="tp", name=f"vtp_{n}_{lo}_{co}"
                        )
                        nc.tensor.transpose(
                            pt, vf_bf[:, lo, bass.ts(co, P)], identity
                        )
                        nc.vector.tensor_copy(vfT_bf[:, co, bass.ts(lo, P)], pt)
            else:
                for lo in range(LO):
                    nc.sync.dma_start_transpose(
                        vfT_bf[:, :, bass.ts(lo, P)], vf_bf[:, lo, :]
                    )
            return vf_bf, vfT_bf

        tiles0 = phase_a(0)

        # ---- weights prep ----
        # W1 [DH, F] -> w1T [f(p), fo, d] bf16 via PE transposes (PE is idle
        # during the prologue; keeps the DMA engines free for v streaming)
        w1T = consts.tile([P, FO, DH], BF16)
        w1vT = w1T[:, :CO, :]
        w1hT = w1T[:, CO:, :]
        for do in range(DO):
            st = wstage.tile([P, F], FP32, tag="w1st")
            nc.sync.dma_start(st, w1_ext[bass.ts(do, P), :])
            stb = wstage.tile([P, F], BF16, tag="w1stb")
            nc.scalar.copy(stb, st)
            for fo in range(FO):
                pt = tpsum.tile([P, P], BF16, tag="tp")
                nc.tensor.transpose(pt, stb[:, bass.ts(fo, P)], identity)
                nc.vector.tensor_copy(w1T[:, fo, bass.ts(do, P)], pt)

        # w2 replicated across partitions, bf16: [p, d]
        w2_st = wstage.tile([P, DH], FP32, tag="w2st", bufs=1)
        nc.sync.dma_start(w2_st, bcast(w2_ext.ap()[0], P))
        w2_rep = consts.tile([P, DH], BF16)
        nc.scalar.copy(w2_rep, w2_st)

        # b1 replicated across the nb partitions: [nb, d] f32
        b1_rep = consts.tile([nb, DH], FP32)
        nc.sync.dma_start(b1_rep, bcast(b1_ext.ap(), nb))

        # ones column for the softmax partition-sum
        ones_col = consts.tile([P, 1], FP32)
        nc.vector.memset(ones_col, 1.0)

        # h [nb, DH] -> hT [dh(p), ho, n] bf16
        h_sb = consts.tile([nb, DH], FP32)
        nc.sync.dma_start(h_sb, h_ext[:, :])
        h_bf = consts.tile([nb, DH], BF16)
        nc.scalar.copy(h_bf, h_sb)
        hT = consts.tile([P, DO, nb], BF16)
        for ho in range(DO):
            pt = tpsum.tile([P, nb], BF16, tag="tp")
            nc.tensor.transpose(pt, h_bf[:, bass.ts(ho, P)], identity[:nb, :nb])
            nc.vector.tensor_copy(hT[:, ho, :], pt)

        # G[n, d] = h[n] @ W1h.T + b1, natural row layout, staged to DRAM so each
        # batch can partition-broadcast its row back cheaply.
        G_nat = consts.tile([nb, DH], FP32)
        for dh in range(2):
            gpt = mpsum.tile([P, 512], FP32, tag="mp", name=f"gpt{dh}")
            gp = gpt[:nb, :]
            for ho in range(DO):
                nc.tensor.matmul(
                    gp,
                    lhsT=hT[:, ho, :],
                    rhs=w1hT[:, ho, bass.ts(dh, 512)],
                    start=(ho == 0),
                    stop=(ho == DO - 1),
                )
            nc.vector.tensor_add(
                G_nat[:, bass.ts(dh, 512)], gp, b1_rep[:, bass.ts(dh, 512)]
            )
        nc.sync.dma_start(gscr[:, :], G_nat[:, :])

        # ---- per-batch main loop ----
        for n in range(nb):
            vf_bf, vfT_bf = tiles0 if n == 0 else phase_a(n)

            # g row for this batch, broadcast to all 128 partitions
            g_rep = spool.tile([P, DH], FP32, tag="grep", name=f"grep_{n}", bufs=2)
            nc.sync.dma_start(g_rep, bcast(gscr[n : n + 1, :], P))

            # Phase B: psum <- g ; psum += vf @ W1v.T ; e = tanh(psum)  [l(p), d]
            e_nat = vpool.tile([P, LO, DH], BF16, tag="e", name=f"e_{n}")
            for lo in range(LO):
                ep = mpsum.tile([P, DH], FP32, tag="mp", name=f"ep_{lo}")
                for dh in range(2):
                    for co in range(CO):
                        nc.tensor.matmul(
                            ep[:, bass.ts(dh, 512)],
                            lhsT=vfT_bf[:, co, bass.ts(lo, P)],
                            rhs=w1vT[:, co, bass.ts(dh, 512)],
                            start=(co == 0),
                            stop=(co == CO - 1),
                        )
                nc.vector.tensor_add(ep, ep, g_rep)
                nc.scalar.activation(e_nat[:, lo, :], ep, func=TANH)

            # Phase C: scores[l] = sum_d w2[d] e[l, d] on DVE; p = exp(scores)
            scol = npool.tile([P, LO], FP32, tag="scol", name=f"scol_{n}")
            for lo in range(LO):
                tmp = spool.tile([P, DH], BF16, tag="wtmp", name=f"wtmp_{n}_{lo}", bufs=2)
                nc.vector.tensor_mul(tmp, e_nat[:, lo, :], w2_rep)
                nc.vector.reduce_sum(
                    scol[:, lo : lo + 1], tmp, axis=mybir.AxisListType.X
                )
            pcol = npool.tile([P, LO], BF16, tag="pcol", name=f"pcol_{n}")
            psum_col = npool.tile([P, 1], FP32, tag="psc", name=f"psc_{n}")
            nc.scalar.activation(pcol, scol, func=EXP, accum_out=psum_col)
            # total = sum over partitions (one tiny f32 matmul), then 1/total
            tot = spsum.tile([1, 1], FP32, tag="sp", name=f"tot_{n}")
            nc.tensor.matmul(
                tot, lhsT=psum_col, rhs=ones_col, start=True, stop=True
            )
            rsum = npool.tile([1, 1], FP32, tag="rsum", name=f"rsum_{n}")
            nc.vector.reciprocal(rsum, tot)

            # Phase D: out[n, c] = rsum * sum_l p[l] vf[l, c]
            out_row = npool.tile([1, C], FP32, tag="orow", name=f"orow_{n}")
            for ch in range(2):
                op = spsum.tile([1, 512], FP32, tag="sp", name=f"op_{n}_{ch}")
                for lo in range(LO):
                    nc.tensor.matmul(
                        op,
                        lhsT=pcol[:, lo : lo + 1],
                        rhs=vf_bf[:, lo, bass.ts(ch, 512)],
                        start=(lo == 0),
                        stop=(lo == LO - 1),
                    )
                nc.vector.tensor_scalar_mul(
                    out_row[:, bass.ts(ch, 512)], op, rsum[:, 0:1]
                )
            nc.sync.dma_start(out_ext[n : n + 1, :], out_row[0:1, :])

    nc.compile()
    return nc


_NC_CACHE: dict[int, bass.Bass] = {}


def _get_nc(nb: int) -> bass.Bass:
    if nb not in _NC_CACHE:
        _NC_CACHE[nb] = build_kernel(nb)
    return _NC_CACHE[nb]


def run(inputs: dict, trace: bool = False):
    """Shard, execute on 8 NeuronCores, gather. Returns (out, BassKernelResults)."""
    h = np.ascontiguousarray(np.asarray(inputs["h"], dtype=np.float32))
    v = np.ascontiguousarray(
        np.asarray(inputs["v"], dtype=np.float32).reshape(N, L, C)
    )
    W1 = np.ascontiguousarray(np.asarray(inputs["W1"], dtype=np.float32))
    b1 = np.ascontiguousarray(np.asarray(inputs["b1"], dtype=np.float32))
    W2 = np.ascontiguousarray(np.asarray(inputs["W2"], dtype=np.float32))
    assert h.shape == (N, DH) and v.shape == (N, L, C)

    nc = _get_nc(NB)
    in_maps = [
        {
            "h": h[i * NB : (i + 1) * NB],
            "v": v[i * NB : (i + 1) * NB],
            "W1": W1,
            "b1": b1,
            "W2": W2,
        }
        for i in range(N_CORES)
    ]
    res = run_bass_kernel_spmd(
        nc,
        in_maps,
        core_ids=list(range(N_CORES)),
        trace=trace,
        trace_cores=list(range(N_CORES)) if trace else None,
        stitch_traces=False,
    )
    out = np.concatenate([res.results[i]["out"] for i in range(N_CORES)], axis=0)
    return out.astype(np.float32), res


def kernel(**inputs) -> np.ndarray:
    out, _ = run(inputs, trace=False)
    return out


# revision 15
# speedup vs baseline: 1.2318x; 1.2318x over previous
"""Attention-pooling kernel for Trainium2, data-parallel over batch on 8 NeuronCores.

Reference computation per batch n:
  vf = v[n].reshape(L, C)
  x  = concat([vf, broadcast(h[n])], axis=1)      # [L, C+Dh]
  e  = tanh(x @ W1.T + b1)                        # [L, Dh]
  s  = e @ W2.T + b2                              # [L, 1]
  a  = softmax(s, axis=0)
  out[n] = sum_l a[l] * vf[l]                     # [C]

Implementation notes:
  - Sharded batch-parallel: 64 batches -> 8 cores x 8 batches, weights replicated.
  - W1 split: x @ W1.T = vf @ W1v.T + h[n] @ W1h.T; the h-term ("g" = W1h h + b1)
    is computed once per batch instead of once per (batch, l) — halves the flops.
  - b2 is dropped: softmax is shift-invariant and b2 cancels exactly.
  - e is produced in natural [l, d] layout (lhsT=vfT, rhs=w1vT); g is DVE-preloaded
    into PSUM and the matmuls accumulate on top, so tanh needs no bias.
  - scores = sum_d w2[d]*e[l, d] is a free-dim weighted reduction on DVE, landing
    directly in the [l(p), lo] layout pooling wants — no score/pT work on the PE.
  - Softmax without max-subtraction: scores are tiny here and exp stays well inside
    f32 range; normalization is folded into the output copy.
  - v is cast to bf16 once; vfT (c-on-partitions) made by xbar DMA transpose in
    steady state, by the TensorEngine for the first two batches (prologue is
    DMA-bound, PE idle).
"""

import os
from contextlib import ExitStack

import numpy as np

import concourse.bass as bass
import concourse.mybir as mybir
import concourse.tile as tile
from concourse import bacc
from concourse.bass_utils import run_bass_kernel_spmd
from concourse.masks import make_identity

N_CORES = 8
N = 64
NB = N // N_CORES  # batches per core
P = 128
L = 1024  # spatial 32*32
C = 1024  # v channels
DH = 1024  # hidden
F = C + DH
FP32 = mybir.dt.float32
BF16 = mybir.dt.bfloat16
TANH = mybir.ActivationFunctionType.Tanh
EXP = mybir.ActivationFunctionType.Exp

LO = L // P
CO = C // P
DO = DH // P
FO = F // P


def build_kernel(nb: int = NB) -> bass.Bass:
    """Build the per-core Bass graph for an nb-batch shard."""
    nc = bacc.Bacc("TRN2", target_bir_lowering=False, debug=False)

    h_ext = nc.declare_dram_parameter("h", [nb, DH], FP32, isOutput=False)
    v_ext = nc.declare_dram_parameter("v", [nb, L, C], FP32, isOutput=False)
    w1_ext = nc.declare_dram_parameter("W1", [DH, F], FP32, isOutput=False)
    b1_ext = nc.declare_dram_parameter("b1", [DH], FP32, isOutput=False)
    w2_ext = nc.declare_dram_parameter("W2", [1, DH], FP32, isOutput=False)
    out_ext = nc.declare_dram_parameter("out", [nb, C], FP32, isOutput=True)

    gscr = nc.dram_tensor("gscr", [nb, DH], FP32)

    def bcast(ap2d, parts):
        """Partition-broadcast AP for a 1D/row DRAM access pattern."""
        ap = ap2d.ap if len(ap2d.shape) == 1 else ap2d.ap[1:]
        return bass.AP(tensor=ap2d.tensor, offset=ap2d.offset, ap=[[0, parts], *ap])

    with tile.TileContext(nc) as tc, ExitStack() as ctx:
        consts = ctx.enter_context(tc.tile_pool(name="consts", bufs=1))
        wstage = ctx.enter_context(tc.tile_pool(name="wstage", bufs=2))
        vpool = ctx.enter_context(tc.tile_pool(name="vpool", bufs=2))
        spool = ctx.enter_context(tc.tile_pool(name="spool", bufs=3))
        npool = ctx.enter_context(tc.tile_pool(name="npool", bufs=2))
        mpsum = ctx.enter_context(tc.tile_pool(name="mpsum", bufs=4, space="PSUM"))
        tpsum = ctx.enter_context(tc.tile_pool(name="tpsum", bufs=2, space="PSUM"))
        spsum = ctx.enter_context(tc.tile_pool(name="spsum", bufs=2, space="PSUM"))

        identity = consts.tile([P, P], BF16)
        make_identity(nc, identity)

        # ---- phase A for n=0, hoisted so v[0] streams in before W1 prep ----
        # For the first two batches the transpose runs on the (otherwise idle)
        # TensorEngine so the prologue DMA stream stays in pure copy mode; from
        # n=2 on, the xbar DMA transpose overlaps fully under the MLP matmuls.
        def phase_a(n):
            vf_bf = vpool.tile([P, LO, C], BF16, tag="vf", name=f"vf_{n}")
            for lo in range(LO):
                st = spool.tile([P, C], FP32, tag="vstage", name=f"vst_{n}_{lo}")
                nc.sync.dma_start(st, v_ext[n, bass.ts(lo, P), :])
                nc.scalar.copy(vf_bf[:, lo, :], st)
            vfT_bf = vpool.tile([P, CO, L], BF16, tag="vfT", name=f"vfT_{n}")
            if n < 2:
                for lo in range(LO):
                    for co in range(CO):
                        pt = tpsum.tile(
                            [P, P], BF16, tag="tp", name=f"vtp_{n}_{lo}_{co}"
                        )
                        nc.tensor.transpose(
                            pt, vf_bf[:, lo, bass.ts(co, P)], identity
                        )
                        nc.vector.tensor_copy(vfT_bf[:, co, bass.ts(lo, P)], pt)
            else:
                for lo in range(LO):
                    nc.sync.dma_start_transpose(
                        vfT_bf[:, :, bass.ts(lo, P)], vf_bf[:, lo, :]
                    )
            return vf_bf, vfT_bf

        tiles0 = phase_a(0)

        # ---- weights prep ----
        # W1 [DH, F] -> w1T [f(p), fo, d] bf16 via PE transposes (PE is idle
        # during the prologue; keeps the DMA engines free for v streaming)
        w1T = consts.tile([P, FO, DH], BF16)
        w1vT = w1T[:, :CO, :]
        w1hT = w1T[:, CO:, :]
        for do in range(DO):
            st = wstage.tile([P, F], FP32, tag="w1st")
            nc.sync.dma_start(st, w1_ext[bass.ts(do, P), :])
            stb = wstage.tile([P, F], BF16, tag="w1stb")
            nc.scalar.copy(stb, st)
            for fo in range(FO):
                pt = tpsum.tile([P, P], BF16, tag="tp")
                nc.tensor.transpose(pt, stb[:, bass.ts(fo, P)], identity)
                nc.vector.tensor_copy(w1T[:, fo, bass.ts(do, P)], pt)

        # w2 replicated across partitions, bf16: [p, d]
        w2_st = wstage.tile([P, DH], FP32, tag="w2st", bufs=1)
        nc.sync.dma_start(w2_st, bcast(w2_ext.ap()[0], P))
        w2_rep = consts.tile([P, DH], BF16)
        nc.scalar.copy(w2_rep, w2_st)

        # b1 replicated across the nb partitions: [nb, d] f32
        b1_rep = consts.tile([nb, DH], FP32)
        nc.sync.dma_start(b1_rep, bcast(b1_ext.ap(), nb))

        # ones column for the softmax partition-sum
        ones_col = consts.tile([P, 1], FP32)
        nc.vector.memset(ones_col, 1.0)

        # h [nb, DH] -> hT [dh(p), ho, n] bf16
        h_sb = consts.tile([nb, DH], FP32)
        nc.sync.dma_start(h_sb, h_ext[:, :])
        h_bf = consts.tile([nb, DH], BF16)
        nc.scalar.copy(h_bf, h_sb)
        hT = consts.tile([P, DO, nb], BF16)
        for ho in range(DO):
            pt = tpsum.tile([P, nb], BF16, tag="tp")
            nc.tensor.transpose(pt, h_bf[:, bass.ts(ho, P)], identity[:nb, :nb])
            nc.vector.tensor_copy(hT[:, ho, :], pt)

        # G[n, d] = h[n] @ W1h.T + b1, natural row layout, staged to DRAM so each
        # batch can partition-broadcast its row back cheaply.
        G_nat = consts.tile([nb, DH], FP32)
        for dh in range(2):
            gpt = mpsum.tile([P, 512], FP32, tag="mp", name=f"gpt{dh}")
            gp = gpt[:nb, :]
            for ho in range(DO):
                nc.tensor.matmul(
                    gp,
                    lhsT=hT[:, ho, :],
                    rhs=w1hT[:, ho, bass.ts(dh, 512)],
                    start=(ho == 0),
                    stop=(ho == DO - 1),
                )
            nc.vector.tensor_add(
                G_nat[:, bass.ts(dh, 512)], gp, b1_rep[:, bass.ts(dh, 512)]
            )
        nc.sync.dma_start(gscr[:, :], G_nat[:, :])

        # ---- per-batch main loop ----
        for n in range(nb):
            vf_bf, vfT_bf = tiles0 if n == 0 else phase_a(n)

            # g row for this batch, broadcast to all 128 partitions
            g_rep = spool.tile([P, DH], FP32, tag="grep", name=f"grep_{n}", bufs=2)
            nc.sync.dma_start(g_rep, bcast(gscr[n : n + 1, :], P))

            # Phase B: psum <- g ; psum += vf @ W1v.T ; e = tanh(psum)  [l(p), d]
            e_nat = vpool.tile([P, LO, DH], BF16, tag="e", name=f"e_{n}")
            for lo in range(LO):
                for dh in range(2):
                    ep = mpsum.tile([P, 512], FP32, tag="mp", name=f"ep_{lo}_{dh}")
                    for co in range(CO):
                        nc.tensor.matmul(
                            ep,
                            lhsT=vfT_bf[:, co, bass.ts(lo, P)],
                            rhs=w1vT[:, co, bass.ts(dh, 512)],
                            start=(co == 0),
                            stop=(co == CO - 1),
                        )
                    nc.vector.tensor_add(ep, ep, g_rep[:, bass.ts(dh, 512)])
                    nc.scalar.activation(
                        e_nat[:, lo, bass.ts(dh, 512)], ep, func=TANH
                    )

            # Phase C: scores[l] = sum_d w2[d] e[l, d] on DVE; p = exp(scores)
            scol = npool.tile([P, LO], FP32, tag="scol", name=f"scol_{n}")
            for lo in range(LO):
                tmp = spool.tile([P, DH], BF16, tag="wtmp", name=f"wtmp_{n}_{lo}", bufs=2)
                nc.vector.tensor_mul(tmp, e_nat[:, lo, :], w2_rep)
                nc.vector.reduce_sum(
                    scol[:, lo : lo + 1], tmp, axis=mybir.AxisListType.X
                )
            pcol = npool.tile([P, LO], BF16, tag="pcol", name=f"pcol_{n}")
            psum_col = npool.tile([P, 1], FP32, tag="psc", name=f"psc_{n}")
            nc.scalar.activation(pcol, scol, func=EXP, accum_out=psum_col)
            # total = sum over partitions (one tiny f32 matmul), then 1/total
            tot = spsum.tile([1, 1], FP32, tag="sp", name=f"tot_{n}")
            nc.tensor.matmul(
                tot, lhsT=psum_col, rhs=ones_col, start=True, stop=True
            )
            rsum = npool.tile([1, 1], FP32, tag="rsum", name=f"rsum_{n}")
            nc.vector.reciprocal(rsum, tot)

            # Phase D: out[n, c] = rsum * sum_l p[l] vf[l, c]
            out_row = npool.tile([1, C], FP32, tag="orow", name=f"orow_{n}")
            for ch in range(2):
                op = spsum.tile([1, 512], FP32, tag="sp", name=f"op_{n}_{ch}")
                for lo in range(LO):
                    nc.tensor.matmul(
                        op,
                        lhsT=pcol[:, lo : lo + 1],
                        rhs=vf_bf[:, lo, bass.ts(ch, 512)],
                        start=(lo == 0),
                        stop=(lo == LO - 1),
                    )
                nc.vector.tensor_scalar_mul(
                    out_row[:, bass.ts(ch, 512)], op, rsum[:, 0:1]
                )
            nc.sync.dma_start(out_ext[n : n + 1, :], out_row[0:1, :])

    nc.compile()
    return nc


_NC_CACHE: dict[int, bass.Bass] = {}


def _get_nc(nb: int) -> bass.Bass:
    if nb not in _NC_CACHE:
        _NC_CACHE[nb] = build_kernel(nb)
    return _NC_CACHE[nb]


def run(inputs: dict, trace: bool = False):
    """Shard, execute on 8 NeuronCores, gather. Returns (out, BassKernelResults)."""
    h = np.ascontiguousarray(np.asarray(inputs["h"], dtype=np.float32))
    v = np.ascontiguousarray(
        np.asarray(inputs["v"], dtype=np.float32).reshape(N, L, C)
    )
    W1 = np.ascontiguousarray(np.asarray(inputs["W1"], dtype=np.float32))
    b1 = np.ascontiguousarray(np.asarray(inputs["b1"], dtype=np.float32))
    W2 = np.ascontiguousarray(np.asarray(inputs["W2"], dtype=np.float32))
    assert h.shape == (N, DH) and v.shape == (N, L, C)

    nc = _get_nc(NB)
    in_maps = [
        {
            "h": h[i * NB : (i + 1) * NB],
            "v": v[i * NB : (i + 1) * NB],
            "W1": W1,
            "b1": b1,
            "W2": W2,
        }
        for i in range(N_CORES)
    ]
    res = run_bass_kernel_spmd(
        nc,
        in_maps,
        core_ids=list(range(N_CORES)),
        trace=trace,
        trace_cores=list(range(N_CORES)) if trace else None,
        stitch_traces=False,
    )
    out = np.concatenate([res.results[i]["out"] for i in range(N_CORES)], axis=0)
    return out.astype(np.float32), res


def kernel(**inputs) -> np.ndarray:
    out, _ = run(inputs, trace=False)
    return out
